# revision 23
# baseline (speedup 1.0000x reference)
"""Trainium2 Bass kernel for DynamicSparseAttention (single-score-pass).

Reference computation (per batch b, head h):
    scores  = Q @ K^T                      [L, S]
    dense   = softmax(scores, axis=-1)
    routing = dense ** 5
    combined = (routing + dense) * 0.5
    sparse  = combined / sum(combined, -1, keepdims=True)
    out     = sparse @ V                   [L, D]

Math.  With p = exp(s - m1) (m1 a loose per-row upper bound on s, applied via
a bf16 ones-row channel in the score matmul), Z = sum_s p, q = p/Z, and
y = Z.q^5 = exp(5(s - m1) - 4.lnZ):
    out = (A + B) / (A + B)[ones-row],  A = Y@[V|1], B = P@[V|1]
(the per-column 1/Z scale cancels between numerator and denominator, so no
rescale is needed anywhere).  Scores are computed ONCE per tile in bf16 hi/lo
split for fp32 accuracy:  s = khi.(qhi+qlo) + klo.qhi - m1.

y per s-tile comes from one of two per-tile paths (mix balances engines):
  E-path (6/16 tiles): s parked to a DRAM scratch during round A (scalar-
      engine copy + DMA), then y = exp(5.(s - 0.8 lnZ)) — the shift applied
      by a fused DVE scalar_tensor_tensor, the exp on the scalar engine.
  P-path (10/16): q = p*zinv; q2 = q^2; q4 = q2^2; y = q4*p  (bf16 DVE muls;
      zinv is broadcast-landed twice at both 4KB bank phases and the slice
      is chosen per tile at build time to dodge SBUF bank conflicts).
All reciprocals are computed as exp(-ln(x)) on the scalar engine (the DVE
reciprocal instruction costs ~6.5us per row here).  Row broadcasts go
through a DRAM scratch and land partition-replicated via stride-0 DMA reads
(this walrus build cannot encode gpsimd partition_broadcast).

Layout: scores transposed [s partitions, l free]; V' = [v|1] stationary;
operands are pre-transposed on the host so no on-chip transposes exist.
Output is written transposed [D, L] per pair; the host gather re-transposes.

Software pipeline: units u = (pair, l-half); while unit u streams its score
matmuls + exp + PV-B (phase A), unit u-1 runs round B (powering / exp5 +
PV-A) and unit u-2 finishes its epilogue.  PSUM: 2 score bufs (4 banks) +
one accB (2) + one accA (2) = 8 banks.  Emission leads (PVB_LEAD/PVA_LEAD/
B_DELAY) keep the PE fed; matmuls are ordered to share LDWEIGHTS.

Sharding: B*H = 32 (b,h) pairs, 4 per core across 8 cores, no cross-core
communication.  kernel() takes full inputs and returns the full output.
Measured: 409.8us HW exec (vs 518.7us baseline), max rel err 3.1e-3.
"""

import os
import sys
import numpy as np

for _p in ("/opt/trn_rl_repo",):
    if os.path.isdir(_p) and _p not in sys.path:
        sys.path.insert(0, _p)

from contextlib import ExitStack

import json as _json

import ml_dtypes

import concourse.bass as bass
import concourse.mybir as mybir
import concourse.tile as tile
import concourse.bass2jax as _bass2jax
import concourse.bass_utils as _bass_utils
from concourse.bass_utils import run_bass_kernel_spmd

# ---------------------------------------------------------------------------
# Workaround: this container's walrus build rejects instructions carrying
# more than one sync wait.  Rewrite the BIR JSON before compilation: excess
# waits are hoisted onto freshly inserted same-engine NoOp instructions
# placed immediately before the instruction, one wait each.
# ---------------------------------------------------------------------------

_MAX_WAITS = 1


def _split_waits_in_bir(bir_json: bytes) -> bytes:
    bir = _json.loads(bir_json)
    n_new = [0]

    def fix_block(bb):
        out = []
        for inst in bb["instructions"]:
            si = inst.get("sync_info") or {}
            waits = si.get("on_wait") or []
            if len(waits) > _MAX_WAITS:
                excess, keep = waits[:-_MAX_WAITS], waits[-_MAX_WAITS:]
                for w in excess:
                    n_new[0] += 1
                    out.append({
                        "debug": inst.get("debug", 0),
                        "engine": inst["engine"],
                        "ins": [],
                        "name": "I-wsplit-%d" % n_new[0],
                        "opcode": "NoOp",
                        "outs": [],
                        "sync_info": {"on_update": [], "on_wait": [w]},
                    })
                si["on_wait"] = keep
            out.append(inst)
        bb["instructions"] = out

    for fn in bir["functions"]:
        for bb in fn["blocks"]:
            fix_block(bb)
    return _json.dumps(bir).encode()


_orig_compile_bir_kernel = _bass_utils.compile_bir_kernel


def _patched_compile_bir_kernel(bir_json, tmpdir, neff_name="file.neff"):
    return _orig_compile_bir_kernel(
        _split_waits_in_bir(bir_json), tmpdir, neff_name=neff_name
    )


_bass_utils.compile_bir_kernel = _patched_compile_bir_kernel
_bass2jax.compile_bir_kernel = _patched_compile_bir_kernel

# ---------------------------------------------------------------------------
# Problem constants / tuning knobs
# ---------------------------------------------------------------------------

B, L, S, H, E, D = 2, 2048, 2048, 16, 64, 64
NCORES = 8
NP = (B * H) // NCORES   # pairs per core = 4
LHALF = 1024             # l columns per unit
NLH = L // LHALF         # 2
NU = NP * NLH            # 8 units per core
ST = S // 128            # 16 s-tiles
CHW = 512                # matmul chunk width (one PSUM bank of fp32)
NCH = LHALF // CHW       # 2
DV = D + 1               # 65: [v | 1]
KB = E + 1               # 65: [klo | 1] / [qhi | -m1]
FACTOR = 5.0

# E-path s-tiles (scalar-engine exp5); the rest use DVE powering.
E_LIST = (2, 5, 8, 11, 13, 15)
GP_Q_SET = frozenset()  # P-tiles whose q-mult runs on gpsimd
E_SET = frozenset(E_LIST)
NE = len(E_LIST)
B_DELAY = 3              # iters between A(u) and interleaved B(u-1) tile ops
PVB_LEAD = 2             # PV-B(u, st) emitted at iter st+PVB_LEAD
PVA_LEAD = 6             # PV-A5(u-1, st) emitted at iter st+PVA_LEAD
NIT = ST + PVA_LEAD      # inner iterations per unit slot
_EPI_AT = {0: 0, 3: 1, 6: 2, 9: 3}  # iter -> epilogue step

F32 = mybir.dt.float32
BF16 = mybir.dt.bfloat16
EXP = mybir.ActivationFunctionType.Exp
LN = mybir.ActivationFunctionType.Ln

M_COEF = float(np.sqrt(2.0 * np.log(S)))
M_MARGIN = 25.0


def _e_idx(st):
    return E_LIST.index(st)


def _emit(ctx: ExitStack, tc: tile.TileContext, qta, qtb, kta, ktb, va, scr,
          zscr, zscrb, outp):
    nc = tc.nc

    oper = ctx.enter_context(tc.tile_pool(name="oper", bufs=2))
    vpool = ctx.enter_context(tc.tile_pool(name="vp", bufs=2))
    ppool = ctx.enter_context(tc.tile_pool(name="pp", bufs=2))
    spool = ctx.enter_context(tc.tile_pool(name="slp", bufs=4))
    qpool = ctx.enter_context(tc.tile_pool(name="qp", bufs=2))
    zpool = ctx.enter_context(tc.tile_pool(name="zp", bufs=1))
    epool = ctx.enter_context(tc.tile_pool(name="ep", bufs=1))

    ps_sc = ctx.enter_context(tc.tile_pool(name="ps_sc", bufs=2, space="PSUM"))
    ps_b = ctx.enter_context(tc.tile_pool(name="ps_b", bufs=1, space="PSUM"))
    ps_a = ctx.enter_context(tc.tile_pool(name="ps_a", bufs=1, space="PSUM"))

    opers = {}   # pair -> dict of operand tiles
    U = {}       # unit -> dict of state

    def load_pair(pair):
        qta_t = oper.tile([128, L], BF16, tag="qta", name="qta_t")
        nc.gpsimd.dma_start(out=qta_t, in_=qta[pair])
        qtb_t = oper.tile([KB, L], BF16, tag="qtb", name="qtb_t")
        nc.gpsimd.dma_start(out=qtb_t, in_=qtb[pair])
        kta_t = oper.tile([128, S], BF16, tag="kta", name="kta_t")
        nc.gpsimd.dma_start(out=kta_t, in_=kta[pair])
        ktb_t = oper.tile([KB, S], BF16, tag="ktb", name="ktb_t")
        nc.gpsimd.dma_start(out=ktb_t, in_=ktb[pair])
        vts = []
        for t in range(ST):
            vt = vpool.tile([128, DV], BF16, tag=f"v{t}", name=f"vt{t}")
            nc.gpsimd.dma_start(out=vt, in_=va[pair, t * 128:(t + 1) * 128, :])
            vts.append(vt)
        opers[pair] = dict(qta=qta_t, qtb=qtb_t, kta=kta_t, ktb=ktb_t,
                           vts=vts)

    def emit_A_mms(u, st):
        """Score matmuls + exp for (u, st); parks s to DRAM for E-tiles."""
        pair, lh = divmod(u, NLH)
        op = opers[pair]
        l0 = lh * LHALF
        sc = ps_sc.tile([128, LHALF], F32, tag="sc", name="sc")
        if st in E_SET:
            p = ppool.tile([128, LHALF], BF16, tag="pE", bufs=PVB_LEAD + 1,
                           name="pE")
        else:
            p = ppool.tile([128, LHALF], BF16, tag=f"pP{st}", bufs=2,
                           name="pP")
        ss = slice(st * 128, (st + 1) * 128)
        for c in range(NCH):
            cs = slice(c * CHW, (c + 1) * CHW)
            gs = slice(l0 + c * CHW, l0 + (c + 1) * CHW)
            nc.tensor.matmul(sc[:, cs], lhsT=op["kta"][:, ss],
                             rhs=op["qta"][:, gs], start=True, stop=False)
        for c in range(NCH):
            cs = slice(c * CHW, (c + 1) * CHW)
            gs = slice(l0 + c * CHW, l0 + (c + 1) * CHW)
            nc.tensor.matmul(sc[:, cs], lhsT=op["ktb"][:, ss],
                             rhs=op["qtb"][:, gs], start=False, stop=True)
        nc.scalar.activation(p, sc, EXP, bias=0.0, scale=1.0)
        if st in E_SET:
            # bounce s through SBUF (gpsimd copy), park in DRAM scratch
            s_tmp = spool.tile([128, LHALF], F32, tag="stmp", bufs=2,
                               name="s_tmp")
            nc.scalar.activation(s_tmp, sc,
                                 mybir.ActivationFunctionType.Copy,
                                 bias=0.0, scale=1.0)
            nc.sync.dma_start(out=scr[u % 2, _e_idx(st)], in_=s_tmp)
        U[u]["p"][st] = p

    def emit_A_pvb(u, st):
        pair, _lh = divmod(u, NLH)
        if st == 0:
            U[u]["accB"] = ps_b.tile([DV, LHALF], F32, tag="accB",
                                     name="accB")
        accB = U[u]["accB"]
        p = U[u]["p"][st]
        vt = opers[pair]["vts"][st]
        for c in range(NCH):
            cs = slice(c * CHW, (c + 1) * CHW)
            nc.tensor.matmul(accB[:, cs], lhsT=vt, rhs=p[:, cs],
                             start=(st == 0), stop=(st == ST - 1))

    def _bcast_land(out_tile, row_ap):
        """DMA-land a parked DRAM row replicated across partitions."""
        nparts = out_tile.shape[0]
        free = row_ap.ap[-1][1]
        bap = bass.AP(row_ap.tensor, row_ap.offset, [[0, nparts], [1, free]])
        nc.sync.dma_start(out=out_tile, in_=bap)

    def emit_mid(u):
        """After PV-B(u,15): Z rows via ACT ln/exp, DMA broadcasts, accB copy."""
        accB = U[u]["accB"]
        b_sb = epool.tile([DV, LHALF], F32, tag="bsb", bufs=2, name="b_sb")
        nc.scalar.activation(b_sb, accB, mybir.ActivationFunctionType.Copy,
                             bias=0.0, scale=1.0)   # frees accB (bufs=1)
        lnz = zpool.tile([1, LHALF], F32, tag="lnz", name="lnz")
        nc.scalar.activation(lnz, accB[D:DV, :], LN, bias=0.0, scale=1.0)
        zi16r = zpool.tile([1, LHALF], BF16, tag="zi16r", name="zi16r")
        nc.scalar.activation(zi16r, lnz, EXP, bias=0.0, scale=-1.0)
        # park rows, land partition-replicated (stride-0 DRAM reads)
        par = u % 2
        nc.sync.dma_start(out=zscrb[par, 0:1, :], in_=zi16r)
        nc.sync.dma_start(out=zscr[par, 1:2, :], in_=lnz)
        zbig = zpool.tile([128, 2 * LHALF], BF16, tag="zbig", name="zbig")
        rap = zscrb[par, 0:1, :]
        bap2 = bass.AP(rap.tensor, rap.offset, [[0, 128], [0, 2], [1, LHALF]])
        nc.sync.dma_start(out=zbig, in_=bap2)
        m2b = zpool.tile([128, LHALF], F32, tag="m2b", name="m2b")
        _bcast_land(m2b, zscr[par, 1:2, :])
        U[u].update(b_sb=b_sb, zbig=zbig, m2b=m2b)

    def emit_landing(u, st):
        """Bring parked s back from DRAM shortly before its round-B sub."""
        sl = spool.tile([128, LHALF], F32, tag="sland", bufs=2, name="sland")
        nc.sync.dma_start(out=sl, in_=scr[u % 2, _e_idx(st)])
        U[u]["sland"][st] = sl

    def emit_B_tile(u, st):
        """Round-B rhs for (u, st): p5 via exp5 (E) or q^5 via powering."""
        if st in E_SET:
            sp = qpool.tile([128, LHALF], F32, tag="sp", bufs=2, name="sp")
            # sp = s - 0.8*lnZ  (rhs is Z*q^5; zinv cancels in the output)
            nc.vector.scalar_tensor_tensor(
                sp, U[u]["m2b"], -0.8, U[u]["sland"][st],
                mybir.AluOpType.mult, mybir.AluOpType.add)
            r5 = qpool.tile([128, LHALF], BF16, tag="p5", bufs=PVA_LEAD - B_DELAY + 2,
                            name="p5")
            nc.scalar.activation(r5, sp, EXP, bias=0.0, scale=FACTOR)
        else:
            p = U[u]["p"][st]
            q = qpool.tile([128, LHALF], BF16, tag="q", bufs=2, name="q")
            off = _PMAP.get((u, st), 0) * LHALF
            _PROBE[(u, st)] = (p.tensor.name, U[u]["zbig"].tensor.name)
            eng = nc.gpsimd if st in GP_Q_SET else nc.vector
            eng.tensor_mul(q, p, U[u]["zbig"][:, off:off + LHALF])
            q2 = qpool.tile([128, LHALF], BF16, tag="q2", bufs=2, name="q2",
                            padded_shape=[128, LHALF + LHALF // 2])
            nc.vector.tensor_mul(q2, q, q)
            q4 = qpool.tile([128, LHALF], BF16, tag="q4", bufs=2, name="q4")
            nc.vector.tensor_mul(q4, q2, q2)
            r5 = qpool.tile([128, LHALF], BF16, tag="q5", bufs=PVA_LEAD - B_DELAY + 2,
                            name="q5")
            nc.vector.tensor_mul(r5, q4, p)
        U[u]["r5"][st] = r5

    def emit_B_pva5(u, st):
        pair, _lh = divmod(u, NLH)
        if st == 0:
            U[u]["accA"] = ps_a.tile([DV, LHALF], F32, tag="accA",
                                     name="accA")
        accA = U[u]["accA"]
        r5 = U[u]["r5"][st]
        vt = opers[pair]["vts"][st]
        for c in range(NCH):
            cs = slice(c * CHW, (c + 1) * CHW)
            nc.tensor.matmul(accA[:, cs], lhsT=vt, rhs=r5[:, cs],
                             start=(st == 0), stop=(st == ST - 1))

    def emit_epi(u, step):
        pair, lh = divmod(u, NLH)
        l0 = lh * LHALF
        st8 = U[u]
        if step == 0:
            n2 = epool.tile([DV, LHALF], F32, tag="n2", bufs=1, name="n2")
            nc.vector.tensor_add(n2, st8["b_sb"], st8["accA"])  # frees accA
            st8["n2"] = n2
        elif step == 1:
            lnd = epool.tile([1, LHALF], F32, tag="lnd", bufs=1, name="lnd")
            nc.scalar.activation(lnd, st8["n2"][D:DV, :], LN, bias=0.0,
                                 scale=1.0)
            dr = epool.tile([1, LHALF], F32, tag="dr", bufs=1, name="dr")
            nc.scalar.activation(dr, lnd, EXP, bias=0.0, scale=-1.0)
            nc.sync.dma_start(out=zscr[u % 2, 2:3, :], in_=dr)
        elif step == 2:
            dinvb = epool.tile([D, LHALF], F32, tag="dinvb", bufs=1,
                               name="dinvb")
            _bcast_land(dinvb, zscr[u % 2, 2:3, :])
            st8["dinvb"] = dinvb
        elif step == 3:
            ot = epool.tile([D, LHALF], F32, tag="ot", bufs=1, name="ot")
            nc.gpsimd.tensor_mul(ot, st8["n2"][0:D, :], st8["dinvb"])
            nc.gpsimd.dma_start(out=outp[pair, :, l0:l0 + LHALF], in_=ot)

    # ---- main pipeline ---------------------------------------------------
    load_pair(0)
    for u in range(NU + 1):
        if u < NU:
            U[u] = dict(p={}, r5={}, sland={})
            pair, lh = divmod(u, NLH)
        for it in range(NIT):
            if u < NU and it < ST:
                emit_A_mms(u, it)
            if u < NU and PVB_LEAD <= it < ST + PVB_LEAD:
                emit_A_pvb(u, it - PVB_LEAD)
            if u >= 1 and (it + 1) in E_SET and it + 1 < ST:
                emit_landing(u - 1, it + 1)
            if u >= 1 and B_DELAY <= it < ST + B_DELAY:
                emit_B_tile(u - 1, it - B_DELAY)
            if u >= 1 and PVA_LEAD <= it:
                emit_B_pva5(u - 1, it - PVA_LEAD)
            if u >= 2 and it in _EPI_AT:
                emit_epi(u - 2, _EPI_AT[it])
            if u < NU and lh == 0 and it == 10 and pair + 1 < NP:
                load_pair(pair + 1)
        if u < NU:
            emit_mid(u)
            if 0 in E_SET or 1 in E_SET or 2 in E_SET:
                for st in E_LIST:
                    if st <= B_DELAY:
                        emit_landing(u, st)
    # epilogue of the last unit
    for step in range(4):
        emit_epi(NU - 1, step)


_CACHE = {}
_PMAP = {}
_PROBE = {}


def _sbuf_addrs(nc):
    out = {}
    for fn in nc.m.functions:
        for mls in fn.allocations:
            if not getattr(mls, "memorylocations", None):
                continue
            ml = mls.memorylocations[0]
            if "SB" in str(getattr(ml, "type", "")):
                out[mls.name] = ml.addr
    return out


def _build_once():
    nc = bass.Bass()
    qta = nc.declare_dram_parameter("qta", [NP, 128, L], BF16, isOutput=False)
    qtb = nc.declare_dram_parameter("qtb", [NP, KB, L], BF16, isOutput=False)
    kta = nc.declare_dram_parameter("kta", [NP, 128, S], BF16, isOutput=False)
    ktb = nc.declare_dram_parameter("ktb", [NP, KB, S], BF16, isOutput=False)
    va = nc.declare_dram_parameter("va", [NP, S, DV], BF16, isOutput=False)
    outp = nc.declare_dram_parameter("out", [NP, D, L], F32, isOutput=True)
    scr = nc.dram_tensor("sscr", [2, NE, 128, LHALF], F32, kind="Internal")
    zscr = nc.dram_tensor("zscr", [2, 3, LHALF], F32, kind="Internal")
    zscrb = nc.dram_tensor("zscrb", [2, 1, LHALF], BF16, kind="Internal")
    with tile.TileContext(nc) as tc:
        with ExitStack() as ctx:
            _emit(ctx, tc, qta[:], qtb[:], kta[:], ktb[:], va[:], scr[:],
                  zscr[:], zscrb[:], outp[:])
    return nc


def _build():
    if "nc" in _CACHE:
        return _CACHE["nc"]
    _PROBE.clear()
    nc = _build_once()
    addrs = _sbuf_addrs(nc)
    want = {}
    for (u, st), (pname, zname) in _PROBE.items():
        pa = addrs.get(pname, addrs.get(pname + "_set"))
        za = addrs.get(zname, addrs.get(zname + "_set"))
        if pa is None or za is None:
            continue
        # pick the zbig half whose 4KB bank class differs from the p tile
        want[(u, st)] = 1 if (pa - za) % 4096 == 0 else 0
    if want != dict(_PMAP):
        _PMAP.clear()
        _PMAP.update(want)
        _PROBE.clear()
        nc = _build_once()
    _CACHE["nc"] = nc
    return nc


def _prep_inputs(queries, keys, values):
    bf = ml_dtypes.bfloat16
    q = np.ascontiguousarray(
        np.asarray(queries, np.float32).transpose(0, 2, 1, 3)
    ).reshape(B * H, L, E)
    k = np.ascontiguousarray(
        np.asarray(keys, np.float32).transpose(0, 2, 1, 3)
    ).reshape(B * H, S, E)
    v = np.ascontiguousarray(
        np.asarray(values, np.float32).transpose(0, 2, 1, 3)
    ).reshape(B * H, S, D)
    qhi = q.astype(bf)
    qlo = (q - qhi.astype(np.float32)).astype(bf)
    khi = k.astype(bf)
    klo = (k - khi.astype(np.float32)).astype(bf)
    m1 = (M_COEF * np.sqrt((q.astype(np.float64) ** 2).sum(-1)) + M_MARGIN
          ).astype(np.float32)  # [BH, L]
    one_s = np.ones((B * H, 1, S), bf)
    qta = np.concatenate([qhi.transpose(0, 2, 1),
                          qlo.transpose(0, 2, 1)], axis=1)          # [.,128,L]
    qtb = np.concatenate([qhi.transpose(0, 2, 1),
                          (-m1[:, None, :]).astype(bf)], axis=1)    # [.,65,L]
    kta = np.concatenate([khi.transpose(0, 2, 1),
                          khi.transpose(0, 2, 1)], axis=1)          # [.,128,S]
    ktb = np.concatenate([klo.transpose(0, 2, 1), one_s], axis=1)   # [.,65,S]
    va = np.concatenate([v.astype(bf), np.ones((B * H, S, 1), bf)],
                        axis=-1)                                    # [.,S,65]
    in_maps = []
    for c in range(NCORES):
        sl = slice(c * NP, (c + 1) * NP)
        in_maps.append({
            "qta": np.ascontiguousarray(qta[sl]),
            "qtb": np.ascontiguousarray(qtb[sl]),
            "kta": np.ascontiguousarray(kta[sl]),
            "ktb": np.ascontiguousarray(ktb[sl]),
            "va": np.ascontiguousarray(va[sl]),
        })
    return in_maps


def _gather(results):
    outs = np.stack([results[c]["out"] for c in range(NCORES)])  # [8,NP,D,L]
    out = outs.reshape(B, H, D, L).transpose(0, 3, 1, 2)  # [B, L, H, D]
    return np.ascontiguousarray(out)


def run_sharded(queries, keys, values, **kw):
    """Run on the 8 neuron cores; returns (full_output, BassKernelResults)."""
    nc = _build()
    in_maps = _prep_inputs(queries, keys, values)
    res = run_bass_kernel_spmd(nc, in_maps, list(range(NCORES)), **kw)
    return _gather(res.results), res


def kernel(queries, keys, values):
    out, _ = run_sharded(queries, keys, values)
    return out
